# revision 3
# baseline (speedup 1.0000x reference)
"""Trainium2 Bass kernel for nn_AtomsNetwork (gnn_message_passing), v3.

Strategy (per core, atoms sharded 2000/core):
  Gathers: non-transpose HBM-source dma_gather (256B/idx) round-robin over 4
  SWDGE queues (cpu-pair parallel desc-gen). Gathered chunks land pairs-on-
  partitions; neighbor means are computed on the TensorEngine with static
  pair->atom one-hot matrices (K=10 stride), then recip-scaled on DVE.
  L1 uses linearity: gather raw atoms (12 feats) from a replicated DRAM
  table and multiply the mean by Wsr1/Wdr1 afterwards -> no signal-table
  AllGathers at all. L2 gathers y directly from the AllGather output in
  DRAM (no SBUF tables anywhere).
  z accumulates in PSUM: Wv + Wr-stream(residues) + Wsr@meanS + Wdr@meanD.
  Residue means: one-hot segment matmuls; partial sums AllReduced.
  Head: rank trick A[i]+B[j]; per-row relu/matmul chain; rows accumulated
  into a [50,400] PSUM via one-hot outer products; single 80KB output DMA.
"""
import sys
import numpy as np

sys.path.insert(0, '/opt/trn_rl_repo')

N_ATOMS = 16000
NC = 8
K = 10
N_RES = 400
ATOM_CAT = 12
BERT_DIM = 1024
DF2 = 64

LOC = N_ATOMS // NC          # 2000
LOCP = 2048
AC = LOCP // 128             # 16
NT = 4                       # 512-wide zps slices
KB = BERT_DIM // 128         # 8
MYR = N_RES // NC            # 50
RC = -(-N_RES // 128)        # 4
ATAB = 16128                 # L1 gather table rows (zero row at 16000)
CH = 2560                    # idxs per gather call (256 atoms)
NCHK = CH // 128             # 20 chunks per call
NSEG = LOCP * K // CH        # 8 calls per list
NBUF = 8
QUEUES = [0, 1, 2, 3]
DEBUG = False


def build_graph():
    from concourse import bass, bacc, mybir
    from concourse.alu_op_type import AluOpType
    f32, bf16, i16 = mybir.dt.float32, mybir.dt.bfloat16, mybir.dt.int16
    AF = mybir.ActivationFunctionType

    nc = bacc.Bacc(num_swdge_queues=4)
    P = lambda n, s, d: nc.declare_dram_parameter(n, s, d, isOutput=False)
    ins = {}
    for p in (1, 2):
        ins[f'atomsHB_{p}'] = P(f'atomsHB_{p}', [ATAB, 128], bf16)
        ins[f'atomsT_{p}'] = P(f'atomsT_{p}', [ATOM_CAT, LOCP], bf16)
        ins[f'residT_{p}'] = P(f'residT_{p}', [KB, 128, LOCP], bf16)
        ins[f'idxA_{p}'] = P(f'idxA_{p}', [128, 2 * LOCP * K // 16], i16)
        ins[f'idxY_{p}'] = P(f'idxY_{p}', [128, 2 * LOCP * K // 16], i16)
        ins[f'recips_{p}'] = P(f'recips_{p}', [2, LOCP], f32)
        ins[f'rids_{p}'] = P(f'rids_{p}', [128, AC], f32)
    ins['Sseg'] = P('Sseg', [128, 10 * 128], bf16)
    for nm, sh in [('Wsv', [128, 128]), ('Wsr2', [128, 128]), ('Wdr2', [128, 128]),
                   ('Wf2', [128, 2 * DF2]), ('Wf3', [DF2, 1]), ('sel', [128, RC * MYR]),
                   ('Wv', [ATOM_CAT, 128]), ('Wsr1', [ATOM_CAT, 128]),
                   ('Wdr1', [ATOM_CAT, 128]), ('Wr', [128, KB * 128])]:
        ins[nm] = P(nm, sh, bf16)
    for nm, sh in [('Wf1t', [128, 256]), ('Wf1b', [128, 256]), ('bf1', [128, 2]),
                   ('bf2', [DF2, 1]), ('bf3r', [128, 1]), ('recip_res', [1, 2 * N_RES]),
                   ('eyeR', [1, MYR * MYR])]:
        ins[nm] = P(nm, sh, f32)
    out_ext = nc.declare_dram_parameter('out', [MYR, N_RES], f32, isOutput=True)

    shardYs = [nc.dram_tensor(f'shardY_{p}', [AC, 128, 128], bf16) for p in (0, 1)]
    fullYs = [nc.dram_tensor(f'fullY_{p}', [NC, AC, 128, 128], bf16, addr_space='Shared')
              for p in (0, 1)]
    rparts = [nc.dram_tensor(f'rpart_{p}', [128, N_RES], f32) for p in (0, 1)]
    rsums = [nc.dram_tensor(f'rsum_{p}', [128, N_RES], f32, addr_space='Shared')
             for p in (0, 1)]

    steps = []
    cnt = {}

    class Tok:
        __slots__ = ('sem', 'n')
        def __init__(s, sem, n): s.sem, s.n = sem, n

    from contextlib import ExitStack
    _es = ExitStack()
    with _es:
        block = _es.enter_context(nc.Block())
        semnames = ['dma', 'dmaS', 'dmaT', 'dmaR0', 'dmaR1', 'dmaU', 'dmaO',
                    'gq0', 'gq1', 'gq2', 'gq3', 'pe', 'v', 'act', 'g', 'cc']
        sems = {n: _es.enter_context(nc.semaphore(n)) for n in semnames}

        T = lambda nm, sh, dt: _es.enter_context(nc.sbuf_tensor(nm, sh, dt))
        idxb = T('idxb', [128, 2, CH], i16)
        gbuf = T('gbuf', [128, NBUF, NCHK, 128], bf16)
        Sbuf = T('Sbuf', [128, 10, 128], bf16)
        meanS = T('meanS', [128, LOCP], bf16)
        meanD = T('meanD', [128, LOCP], bf16)
        atomsTS = T('atomsTS', [ATOM_CAT, 2 * LOCP], bf16)
        resb = T('resb', [128, 2, LOCP], bf16)
        ybuf = T('ybuf', [128, 2, LOCP], bf16)
        stripesY = T('stripesY', [128, AC, 128], bf16)
        wbuf = T('wbuf', [128, LOCP], bf16)
        rcpSB = [T('rcpSBs', [1, 2 * LOCP], f32), T('rcpSBd', [1, 2 * LOCP], f32)]
        ridsb = T('ridsb', [128, 2 * AC], f32)
        rows16 = T('rows16', [128, 2, 128], bf16)
        Mbuf = T('Mbuf', [128, 2, N_RES], bf16)
        rowsR = T('rowsR', [128, RC, 128], bf16)
        rbuf = T('rbuf', [128, 2, N_RES], f32)
        r1my = T('r1my', [128, MYR], f32)
        Abuf = T('Abuf', [128, 2 * MYR], f32)
        Bbuf = T('Bbuf', [128, 2, N_RES], bf16)
        Xbuf = T('Xbuf', [128, 2, 2, N_RES], bf16)
        h2b = T('h2b', [DF2, 2, N_RES], bf16)
        outrow = T('outrow', [1, 2, N_RES], f32)
        outSB = T('outSB', [MYR, N_RES], f32)
        dbgZ = T('dbgZ', [128, LOCP], f32) if DEBUG else None
        iotaP = T('iotaP', [128, 128], f32)
        idPbf = T('idPbf', [128, 128], bf16)
        iotaR = T('iotaR', [128, N_RES], f32)
        ones1 = T('ones1', [1, 128], f32)
        wWv = T('wWv', [ATOM_CAT, 128], bf16)
        wWsr1 = T('wWsr1', [ATOM_CAT, 128], bf16)
        wWdr1 = T('wWdr1', [ATOM_CAT, 128], bf16)
        wWr = T('wWr', [128, KB * 128], bf16)
        wWsv = T('wWsv', [128, 128], bf16)
        wWsr2 = T('wWsr2', [128, 128], bf16)
        wWdr2 = T('wWdr2', [128, 128], bf16)
        wWf1t = T('wWf1t', [128, 256], f32)
        wWf1b = T('wWf1b', [128, 256], f32)
        wWf2 = T('wWf2', [128, 2 * DF2], bf16)
        wWf3 = T('wWf3', [DF2, 1], bf16)
        wbf1 = T('wbf1', [128, 2], f32)
        wbf2 = T('wbf2', [DF2, 1], f32)
        wbf3r = T('wbf3r', [128, 1], f32)
        wrr = T('wrr', [1, 2 * N_RES], f32)
        wsel = T('wsel', [128, RC * MYR], bf16)
        weyeR = T('weyeR', [1, MYR * MYR], f32)

        def S(eng, emit, waits=(), inc=None, amt=1):
            _m = {}
            for t in waits:
                if t is not None and _m.get(id(t.sem), (None, -1))[1] < t.n:
                    _m[id(t.sem)] = (t.sem, t.n)
            cw = list(_m.values())
            semobj = sems[inc] if inc else None
            def fn(e, cw=cw, emit=emit, semobj=semobj, amt=amt):
                for sm, n in cw:
                    e.wait_ge(sm, n)
                r = emit(e)
                if semobj is not None:
                    r.then_inc(semobj, amt)
            steps.append((eng, fn))
            if inc:
                cnt[inc] = cnt.get(inc, 0) + amt
                return Tok(sems[inc], cnt[inc])
            return None

        zps = nc.place_psum_tensor('zps', [128, LOCP], f32, bank=0)
        trA = nc.place_psum_tensor('trA', [128, 128], bf16, bank=0)
        trB = nc.place_psum_tensor('trB', [128, 128], bf16, bank=1)
        segps = nc.place_psum_tensor('segps', [128, N_RES], f32, bank=2)
        rr2ps = nc.place_psum_tensor('rr2ps', [128, N_RES], f32, bank=2)
        outPS = nc.place_psum_tensor('outPS', [128, N_RES], f32, bank=3)
        mps = [nc.place_psum_tensor(f'mps{i}', [128, 128], f32, bank=4 + i)
               for i in (0, 1)]
        h2ps = [nc.place_psum_tensor(f'h2ps{i}', [DF2, N_RES], f32, bank=4 + i)
                for i in (0, 1)]
        rcpps = [nc.place_psum_tensor(f'rcpps{i}', [128, 256], f32, bank=6 + i)
                 for i in (0, 1)]
        r1ps = nc.place_psum_tensor('r1ps', [128, MYR], f32, bank=6)
        trR = nc.place_psum_tensor('trR', [128, 128], f32, bank=7)
        rr1ps = nc.place_psum_tensor('rr1ps', [128, N_RES], f32, bank=7)
        h3ps = [nc.place_psum_tensor(f'h3ps{i}', [1, N_RES], f32, bank=6 + i)
                for i in (0, 1)]
        Bps = [nc.place_psum_tensor(f'Bps{i}', [128, N_RES], f32, bank=0 + i)
               for i in (0, 1)]

        D = lambda out, in_: (lambda e: e.dma_start(out=out, in_=in_))

        # ---------- phase 0 ----------
        t_idx = [None, None]
        t_idx[0] = S('sync', D(idxb[:, 0, :], ins['idxA_1'][:]), inc='dmaT', amt=16)
        t_idx[1] = S('sync', D(idxb[:, 1, :], ins['idxA_2'][:]), inc='dmaT', amt=16)
        t_dma = None
        for nm, dst in [('Wv', wWv), ('Wsr1', wWsr1), ('Wdr1', wWdr1), ('Wr', wWr),
                        ('Wsv', wWsv), ('Wsr2', wWsr2), ('Wdr2', wWdr2),
                        ('Wf1t', wWf1t), ('Wf1b', wWf1b), ('Wf2', wWf2),
                        ('Wf3', wWf3), ('bf1', wbf1), ('bf2', wbf2), ('bf3r', wbf3r),
                        ('recip_res', wrr), ('sel', wsel), ('Sseg', Sbuf),
                        ('eyeR', weyeR)]:
            t_dma = S('sync', D(dst[:], ins[nm][:]), inc='dma', amt=16)
        t_rcp = [None, None]
        for p in (1, 2):
            t_dma = S('sync', D(atomsTS[:, (p - 1) * LOCP:p * LOCP],
                                ins[f'atomsT_{p}'][:]), inc='dma', amt=16)
            t_dma = S('sync', D(ridsb[:, (p - 1) * AC:p * AC], ins[f'rids_{p}'][:]),
                      inc='dma', amt=16)
            tr0 = S('sync', D(rcpSB[0][:, (p - 1) * LOCP:p * LOCP],
                             ins[f'recips_{p}'][0:1, :]), inc='dma', amt=16)
            t_rcp[p - 1] = S('sync', D(rcpSB[1][:, (p - 1) * LOCP:p * LOCP],
                                       ins[f'recips_{p}'][1:2, :]),
                             waits=[tr0], inc='dma', amt=16)
            t_dma = t_rcp[p - 1]

        t_io = S('g', lambda e: e.iota(iotaP[:], [[1, 128]], channel_multiplier=-1,
                                       allow_small_or_imprecise_dtypes=True), inc='g')
        t_ir = S('g', lambda e: e.iota(iotaR[:], [[1, N_RES]], channel_multiplier=0,
                                       allow_small_or_imprecise_dtypes=True),
                 waits=[t_io], inc='g')
        t_id = S('v', lambda e: e.tensor_scalar(out=iotaP[:], in0=iotaP[:], scalar1=0.0,
                                                scalar2=None, op0=AluOpType.is_equal),
                 waits=[t_io], inc='v')
        t_idb = S('v', lambda e: e.tensor_copy(idPbf[:], iotaP[:]), waits=[t_id], inc='v')
        t_ones = S('v', lambda e: e.memset(ones1[:], 1.0), inc='v')

        gq = ['gq0', 'gq1', 'gq2', 'gq3']
        state = {
            'call': 0,
            'buf_free': [None] * NBUF,
            'mps_free': [None, None],
            'rcp_free': [None, None],
            'meanmm': {0: None, 1: None},
            'tr': [None, None],
            'zfree': None,
            'res_last': None,
        }

        def z_pre(p, layer):
            """Wv/Wsv (start=True) + Wr stream. Emitted BEFORE the gathers."""
            zf = state['zfree']
            for nt in range(NT):
                n0, n1 = nt * 512, (nt + 1) * 512
                if layer == 0:
                    S('pe', (lambda e, n0=n0, n1=n1, p=p:
                             e.matmul(zps[:, n0:n1], wWv[:],
                                      atomsTS[:, p * LOCP + n0:p * LOCP + n1],
                                      start=True, stop=False)),
                      waits=[t_dma, zf], inc='pe')
                else:
                    S('pe', (lambda e, n0=n0, n1=n1, p=p:
                             e.matmul(zps[:, n0:n1], wWsv[:],
                                      ybuf[:, p, n0:n1],
                                      start=True, stop=False)),
                      waits=[t_dma, state['yrelu'][p], zf], inc='pe')
            if layer == 0:
                tres_prev = [None, None]
                guard = state['res_last']
                for kb in range(KB):
                    bslot = kb % 2
                    trd = S('sync', (lambda e, kb=kb, bslot=bslot, p=p: e.dma_start(
                                out=resb[:, bslot, :], in_=ins[f'residT_{p + 1}'][kb])),
                            waits=[tres_prev[0], guard if kb < 2 else None],
                            inc=f'dmaR{bslot}', amt=16)
                    lmm = None
                    for nt in range(NT):
                        n0, n1 = nt * 512, (nt + 1) * 512
                        lmm = S('pe', (lambda e, kb=kb, n0=n0, n1=n1, bslot=bslot:
                                       e.matmul(zps[:, n0:n1],
                                                wWr[:, kb * 128:(kb + 1) * 128],
                                                resb[:, bslot, n0:n1],
                                                start=False, stop=False)),
                                waits=[trd], inc='pe')
                    tres_prev = [tres_prev[1], lmm]
                state['res_last'] = tres_prev[1]

        def gather_pass(p, layer, tok_table, tok_idx, mid=None):
            side_scale = [None, None]
            for c16 in range(2 * NSEG):
                if c16 == NSEG and mid is not None:
                    mid()
                side, q8 = c16 // NSEG, c16 % NSEG
                c = state['call']
                state['call'] += 1
                b = c % NBUF
                qi = QUEUES[c % len(QUEUES)]
                w = [tok_table, tok_idx, state['buf_free'][b]]
                src = (ins[f'atomsHB_{p + 1}'][:] if layer == 0 else
                       fullYs[p][:].rearrange('r s t e -> (r s t) e'))
                tg = S('g', (lambda e, src=src, b=b, p=p, c16=c16, qi=qi:
                             e.dma_gather(
                                 out_ap=gbuf[:, b, :, :],
                                 in_ap=src,
                                 idxs_ap=idxb[:, p, c16 * (CH // 16):(c16 + 1) * (CH // 16)],
                                 num_idxs=CH, num_idxs_reg=CH,
                                 elem_size=128, transpose=False,
                                 single_packet=False,
                                 queue_num=qi)),
                        waits=w, inc=gq[qi], amt=16)
                rp = c % 2
                a0 = q8 * 256
                trc = S('pe', (lambda e, rp=rp, side=side, p=p, a0=a0:
                               e.matmul(rcpps[rp][:], ones1[:],
                                        rcpSB[side][:, p * LOCP + a0:p * LOCP + a0 + 256],
                                        start=True, stop=True)),
                        waits=[t_ones, t_rcp[p], state['rcp_free'][rp]], inc='pe')
                lhs_hi = 16 if layer == 0 else 128
                for g in range(2):
                    G = c * 2 + g
                    mb = G % 2
                    mdst = mps[mb]
                    tm = None
                    for cc10 in range(10):
                        ch = g * 10 + cc10
                        tm = S('pe', (lambda e, mdst=mdst, b=b, ch=ch, cc10=cc10,
                                      lhs_hi=lhs_hi:
                                      e.matmul(mdst[0:lhs_hi, :],
                                               gbuf[:, b, ch, 0:lhs_hi],
                                               Sbuf[:, cc10, :],
                                               start=(cc10 == 0), stop=(cc10 == 9))),
                               waits=([tg, t_dma, state['mps_free'][mb]]
                                      if cc10 == 0 else []), inc='pe')
                    a0g = q8 * 256 + g * 128
                    dst = meanS if side == 0 else meanD
                    tcp = S('act', (lambda e, dst=dst, a0g=a0g, mdst=mdst,
                                    lhs_hi=lhs_hi:
                                    e.activation(dst[0:lhs_hi, a0g:a0g + 128],
                                                 mdst[0:lhs_hi, :], AF.Copy)),
                            waits=[tm, state['meanmm'][side]], inc='act')
                    tsc = S('v', (lambda e, dst=dst, a0g=a0g, rp=rp, g=g,
                                  lhs_hi=lhs_hi:
                                  e.tensor_tensor(
                                      out=dst[0:lhs_hi, a0g:a0g + 128],
                                      in0=dst[0:lhs_hi, a0g:a0g + 128],
                                      in1=rcpps[rp][0:lhs_hi, g * 128:(g + 1) * 128],
                                      op=AluOpType.mult)),
                            waits=[tcp, trc], inc='v')
                    state['mps_free'][mb] = tcp
                    if g == 1:
                        state['rcp_free'][rp] = tsc
                    side_scale[side] = tsc
                state['buf_free'][b] = tm
            return side_scale

        def z_post(p, layer, side_scale):
            WS = wWsr1 if layer == 0 else wWsr2
            WD = wWdr1 if layer == 0 else wWdr2
            hi = ATOM_CAT if layer == 0 else 128
            mmS = mmD = None
            for nt in range(NT):
                n0, n1 = nt * 512, (nt + 1) * 512
                mmS = S('pe', (lambda e, n0=n0, n1=n1, WS=WS, hi=hi:
                               e.matmul(zps[:, n0:n1], WS[:], meanS[0:hi, n0:n1],
                                        start=False, stop=False)),
                        waits=[side_scale[0], t_dma], inc='pe')
            for nt in range(NT):
                n0, n1 = nt * 512, (nt + 1) * 512
                mmD = S('pe', (lambda e, n0=n0, n1=n1, WD=WD, hi=hi:
                               e.matmul(zps[:, n0:n1], WD[:], meanD[0:hi, n0:n1],
                                        start=False, stop=True)),
                        waits=[side_scale[1]], inc='pe')
            state['meanmm'][0] = mmS
            state['meanmm'][1] = mmD
            return mmD

        # ================= L1 =================
        state['yrelu'] = [None, None]
        y_cc = [None, None]
        t_shY = [None, None]

        def mk_ycc(p):
            def mid():
                y_cc[p] = S('g', (lambda e, p=p: e.collective_compute(
                            'AllGather', mybir.AluOpType.bypass,
                            replica_groups=[list(range(NC))],
                            ins=[shardYs[p][:]], outs=[fullYs[p][:]])),
                        waits=[t_shY[p]], inc='cc')
            return mid

        for p in (0, 1):
            z_pre(p, 0)
            sc = gather_pass(p, 0, None, t_idx[p],
                             mid=mk_ycc(0) if p == 1 else None)
            if DEBUG and p == 0:
                od1 = nc.declare_dram_parameter('dbg_meanS1', [128, LOCP], bf16,
                                                isOutput=True)
                tdm1 = S('sync', D(od1[:], meanS[:]), waits=[sc[0]], inc='dmaO', amt=16)
                tzc = S('act', (lambda e: e.activation(dbgZ[:], zps[:], AF.Copy)),
                        waits=[state['res_last']], inc='act')
                od2 = nc.declare_dram_parameter('dbg_z1pre', [128, LOCP], f32,
                                                isOutput=True)
                tdm2 = S('sync', D(od2[:], dbgZ[:]), waits=[tzc], inc='dmaO', amt=16)
                sc = [Tok(sems['dmaO'], cnt['dmaO']), sc[1]]
            tstop = z_post(p, 0, sc)
            tY = S('act', (lambda e, p=p: e.activation(ybuf[:, p, :], zps[:], AF.Relu)),
                   waits=[tstop], inc='act')
            state['yrelu'][p] = tY
            tcps = []
            for c in range(AC):
                tp = (trA, trB)[c % 2]
                tk = S('pe', (lambda e, tp=tp, c=c, p=p:
                              e.transpose(tp[:], ybuf[:, p, c * 128:(c + 1) * 128],
                                          idPbf[:])),
                       waits=[tY, t_idb, state['tr'][c % 2]], inc='pe')
                tc = S('v', (lambda e, tp=tp, c=c:
                             e.tensor_copy(stripesY[:, c, :], tp[:])),
                       waits=[tk] + ([state.get('stripes_free')] if c == 0 and
                                     state.get('stripes_free') else []), inc='v')
                state['tr'][c % 2] = tc
                tcps.append(tc)
            state['zfree'] = state['tr'][1]
            t_shY[p] = S('sync', (lambda e, p=p: e.dma_start(
                        out=shardYs[p][:].rearrange('s t e -> t s e'),
                        in_=stripesY[:])),
                    waits=tcps, inc='dmaS', amt=16)
            state['stripes_free'] = t_shY[p]
            gdone = [Tok(sems[gq[i]], cnt.get(gq[i], 0)) for i in range(4)]
            t_idx[p] = S('sync', (lambda e, p=p: e.dma_start(
                        out=idxb[:, p, :], in_=ins[f'idxY_{p + 1}'][:])),
                     waits=gdone, inc='dmaT', amt=16)

        dbg_toks = {}
        if DEBUG:
            for nm, src_ap, wtok in [
                    ('dbg_meanS', meanS[:], state['meanmm'][0]),
                    ('dbg_meanD', meanD[:], state['meanmm'][1]),
                    ('dbg_y1', ybuf[:, 0, :], state['yrelu'][0]),
                    ('dbg_y2', ybuf[:, 1, :], state['yrelu'][1])]:
                od = nc.declare_dram_parameter(nm, [128, src_ap.shape[-1]],
                                               src_ap.dtype, isOutput=True)
                tk = S('sync', D(od[:], src_ap), waits=[wtok], inc='dmaO', amt=16)
                dbg_toks[nm] = tk
            state['meanmm'][0] = dbg_toks['dbg_meanS']
            state['meanmm'][1] = dbg_toks['dbg_meanD']

        # ================= L2 =================
        def seg_stage(p, tW):
            segs = []
            tseg = None
            for c in range(AC):
                mb = c % 2
                tp = (trA, trB)[mb]
                tk = S('pe', (lambda e, tp=tp, c=c:
                              e.transpose(tp[:], wbuf[:, c * 128:(c + 1) * 128],
                                          idPbf[:])),
                       waits=[tW, t_idb, state['tr'][mb]], inc='pe')
                trow = S('v', (lambda e, tp=tp, mb=mb:
                               e.tensor_copy(rows16[:, mb, :], tp[:])),
                         waits=[tk] + ([segs[-2]] if len(segs) >= 2 else []), inc='v')
                state['tr'][mb] = trow
                tM = S('v', (lambda e, c=c, p=p, mb=mb:
                             e.tensor_scalar(out=Mbuf[:, mb, :],
                                             in0=iotaR[:],
                                             scalar1=ridsb[:, p * AC + c:p * AC + c + 1],
                                             scalar2=None,
                                             op0=AluOpType.is_equal)),
                       waits=[t_ir, t_dma] + ([segs[-2]] if len(segs) >= 2 else []),
                       inc='v')
                tseg = S('pe', (lambda e, mb=mb, c=c:
                                e.matmul(segps[:], rows16[:, mb, :],
                                         Mbuf[:, mb, :],
                                         start=(c == 0), stop=(c == AC - 1))),
                         waits=[trow, tM], inc='pe')
                segs.append(tseg)
            tr_copy = S('v', (lambda e, p=p:
                              e.tensor_copy(rbuf[:, p, :], segps[:])),
                        waits=[tseg], inc='v')
            state['zfree'] = tr_copy
            tup = S('sync', (lambda e, p=p: e.dma_start(
                        out=rparts[p][:], in_=rbuf[:, p, :])),
                    waits=[tr_copy], inc='dmaU', amt=16)
            return tup

        def emit_ar(p, tup):
            tcc = S('g', (lambda e, p=p: e.collective_compute(
                        'AllReduce', mybir.AluOpType.add,
                        replica_groups=[list(range(NC))],
                        ins=[rparts[p][:]], outs=[rsums[p][:]])),
                    waits=[tup], inc='cc')
            tdn = S('sync', (lambda e, p=p: e.dma_start(
                        out=rbuf[:, p, :], in_=rsums[p][:])),
                    waits=[tcc], inc='dmaU', amt=16)
            return tdn

        # L2 p=0
        z_pre(0, 1)
        sc = gather_pass(0, 1, y_cc[0], t_idx[0], mid=mk_ycc(1))
        tstop = z_post(0, 1, sc)
        tW1 = S('act', (lambda e: e.activation(wbuf[:], zps[:], AF.Relu)),
                waits=[tstop], inc='act')
        if DEBUG:
            od = nc.declare_dram_parameter('dbg_w1', [128, LOCP], bf16, isOutput=True)
            dbg_toks['dbg_w1'] = S('sync', D(od[:], wbuf[:]), waits=[tW1],
                                   inc='dmaO', amt=16)
        tup0 = seg_stage(0, tW1)
        ar_box = {}

        def mid_ar0():
            ar_box['dn0'] = emit_ar(0, tup0)

        # L2 p=1
        z_pre(1, 1)
        sc = gather_pass(1, 1, y_cc[1], t_idx[1], mid=mid_ar0)
        tstop = z_post(1, 1, sc)
        tW2 = S('act', (lambda e: e.activation(wbuf[:], zps[:], AF.Relu)),
                waits=[tstop, dbg_toks.get('dbg_w1')], inc='act')

        # rr scaling p0 + rowsel (overlap w2's segment stage)
        ar_dn0 = ar_box['dn0']
        trr1 = S('pe', (lambda e: e.matmul(rr1ps[:], ones1[:],
                    wrr[:, 0:N_RES], start=True, stop=True)),
                waits=[t_ones, t_dma, ar_dn0,
                       state['rcp_free'][0], state['rcp_free'][1]], inc='pe')
        tm1 = S('v', (lambda e: e.tensor_tensor(
                    out=rbuf[:, 0, :], in0=rbuf[:, 0, :], in1=rr1ps[:],
                    op=AluOpType.mult)),
               waits=[trr1, ar_dn0], inc='v')
        if DEBUG:
            od = nc.declare_dram_parameter('dbg_r1', [128, N_RES], f32, isOutput=True)
            dbg_toks['dbg_r1'] = S('sync', D(od[:], rbuf[:, 0, :]), waits=[tm1],
                                   inc='dmaO', amt=16)
        rowsel = []
        tprev = tm1
        for c in range(RC):
            n0 = c * 128
            nres = min(128, N_RES - n0)
            tk = S('pe', (lambda e, c=c, n0=n0, nres=nres:
                          e.transpose(trR[0:nres, :], rbuf[:, 0, n0:n0 + nres],
                                      iotaP[:])),
                   waits=[tm1, t_id, tprev], inc='pe')
            tc = S('v', (lambda e, c=c: e.tensor_copy(rowsR[:, c, :], trR[:])),
                   waits=[tk], inc='v')
            tprev = tc
            rowsel.append(tc)
        rowsel_last = tprev
        tsel = None
        for c in range(RC):
            nres = min(128, N_RES - c * 128)
            tsel = S('pe', (lambda e, c=c, nres=nres:
                            e.matmul(r1ps[:], rowsR[0:nres, c, :],
                                     wsel[0:nres, c * MYR:(c + 1) * MYR],
                                     start=(c == 0), stop=(c == RC - 1))),
                     waits=[rowsel[c], t_dma, state['rcp_free'][0]], inc='pe')
        t_r1my = S('v', lambda e: e.tensor_copy(r1my[:], r1ps[:]),
                   waits=[tsel], inc='v')

        # w2 segment stage
        ar_dn1 = emit_ar(1, seg_stage(1, tW2))

        trr2 = S('pe', (lambda e: e.matmul(rr2ps[:], ones1[:],
                    wrr[:, N_RES:2 * N_RES], start=True, stop=True)),
                waits=[t_ones, ar_dn1, state['zfree']], inc='pe')
        tm2 = S('v', (lambda e: e.tensor_tensor(
                    out=rbuf[:, 1, :], in0=rbuf[:, 1, :], in1=rr2ps[:],
                    op=AluOpType.mult)),
               waits=[trr2, ar_dn1], inc='v')

        # ---------- head ----------
        tA, tB = [], []
        for h in (0, 1):
            tk = S('pe', (lambda e, h=h:
                          e.matmul(Bps[h][:, 0:MYR], wWf1t[:, h * 128:(h + 1) * 128],
                                   r1my[:], start=True, stop=True)),
                   waits=[t_r1my, t_dma, state['tr'][h]], inc='pe')
            tA.append(S('v', (lambda e, h=h: e.tensor_scalar(
                            out=Abuf[:, h * MYR:(h + 1) * MYR], in0=Bps[h][:, 0:MYR],
                            scalar1=wbf1[:, h:h + 1], scalar2=None,
                            op0=AluOpType.add)),
                        waits=[tk], inc='v'))
        for h in (0, 1):
            tk = S('pe', (lambda e, h=h:
                          e.matmul(Bps[h][:], wWf1b[:, h * 128:(h + 1) * 128],
                                   rbuf[:, 1, :], start=True, stop=True)),
                   waits=[tm2] + tA, inc='pe')
            tB.append(S('v', (lambda e, h=h: e.tensor_copy(
                            Bbuf[:, h, :], Bps[h][:])),
                        waits=[tk], inc='v'))

        t_prev = {}
        for i in range(MYR):
            pb = i % 2
            tx0 = S('v', (lambda e, i=i, pb=pb: e.tensor_scalar(
                        out=Xbuf[:, pb, 0, :], in0=Bbuf[:, 0, :],
                        scalar1=Abuf[:, i:i + 1],
                        scalar2=0.0, op0=AluOpType.add, op1=AluOpType.max)),
                    waits=[tB[0], tA[0], t_prev.get(('h2', i - 2))], inc='v')
            tx1 = S('act', (lambda e, i=i, pb=pb: e.activation(
                        Xbuf[:, pb, 1, :], Bbuf[:, 1, :], AF.Relu,
                        bias=Abuf[:, MYR + i:MYR + i + 1])),
                    waits=[tB[1], tA[1], t_prev.get(('h2', i - 2))], inc='act')
            tm = S('pe', (lambda e, pb=pb: e.matmul(h2ps[pb][:], wWf2[:, 0:DF2],
                                                    Xbuf[:, pb, 0, :],
                                                    start=True, stop=False)),
                   waits=[tx0,
                          t_prev.get(('h2', i - 2)),
                          state['mps_free'][pb] if i < 2 else None], inc='pe')
            tm = S('pe', (lambda e, pb=pb: e.matmul(h2ps[pb][:], wWf2[:, DF2:2 * DF2],
                                                    Xbuf[:, pb, 1, :],
                                                    start=False, stop=True)),
                   waits=[tx1], inc='pe')
            th2 = S('act', (lambda e, pb=pb: e.activation(h2b[:, pb, :], h2ps[pb][:],
                                                          AF.Relu, bias=wbf2[:])),
                    waits=[tm, t_prev.get(('m3', i - 2))], inc='act')
            t_prev[('h2', i)] = th2
            tm3 = S('pe', (lambda e, pb=pb: e.matmul(h3ps[pb][:], wWf3[:],
                                                     h2b[:, pb, :],
                                                     start=True, stop=True)),
                    waits=[th2,
                           t_r1my if i < 2 else None,
                           rowsel_last if i < 2 else None,
                           tm1 if i < 2 else None,
                           t_prev.get(('or', i - 2))], inc='pe')
            t_prev[('m3', i)] = tm3
            t_or = S('v', (lambda e, i=i, pb=pb: e.tensor_scalar(
                          out=outrow[:, pb, :], in0=h3ps[pb][:],
                          scalar1=wbf3r[0:1, :], scalar2=None,
                          op0=AluOpType.add)),
                      waits=[tm3, t_prev.get(('od', i - 2))], inc='v')
            t_prev[('or', i)] = t_or
            tod = S('act', (lambda e, i=i, pb=pb: e.dma_start(
                          out=out_ext[i:i + 1, :], in_=outrow[:, pb, :])),
                      waits=[t_or], inc='dmaO', amt=16)
            t_prev[('od', i)] = tod
        S('sync', lambda e: e.nop(),
          waits=[t_prev[('od', MYR - 1)], t_prev[('od', MYR - 2)]])

        @block.sync
        def _(e):
            for eng, fn in steps:
                if eng == 'sync':
                    fn(e)

        @block.tensor
        def _(e):
            for eng, fn in steps:
                if eng == 'pe':
                    fn(e)

        @block.vector
        def _(e):
            for eng, fn in steps:
                if eng == 'v':
                    fn(e)

        @block.scalar
        def _(e):
            for eng, fn in steps:
                if eng == 'act':
                    fn(e)

        @block.gpsimd
        def _(e):
            for eng, fn in steps:
                if eng == 'g':
                    fn(e)

    nc.finalize()
    return nc


def _bf(x):
    import ml_dtypes
    return np.asarray(x, np.float32).astype(ml_dtypes.bfloat16)


def prep_inputs(inputs):
    f32 = np.float32
    NID = 2 * LOCP * K

    def wrap_idx(flat):
        w = flat.reshape(NID // 16, 16).T.astype(np.int16)
        return np.tile(w, (8, 1))

    def mk_idx(same, diff, core, layer):
        lo = core * LOC
        parts = []
        for idx in (same, diff):
            sl = np.asarray(idx)[lo:lo + LOC].astype(np.int64)
            if layer == 0:
                s = np.where(sl < 0, 16000, sl)
                zs = 16000
            else:
                rank, locl = sl // LOC, sl % LOC
                s = np.where(sl < 0, 2000, rank * LOCP + locl)
                zs = 2000
            pad = np.full((LOCP - LOC, K), zs, np.int64)
            s = np.concatenate([s, pad], 0).reshape(-1)
            parts.append(s)
        flat = np.concatenate(parts)
        assert flat.max() < 32768
        return wrap_idx(flat)

    def mk_recips(same, diff, core):
        lo = core * LOC
        out = np.zeros((2, LOCP), f32)
        for t, idx in ((0, same), (1, diff)):
            m = (np.asarray(idx)[lo:lo + LOC] > -1).sum(1)
            out[t, :LOC] = 1.0 / np.maximum(m, 1)
        return out

    cnt_res = [np.zeros(N_RES, f32), np.zeros(N_RES, f32)]
    for p, rid in ((0, inputs['res_ids1']), (1, inputs['res_ids2'])):
        ids, c = np.unique(np.asarray(rid), return_counts=True)
        cnt_res[p][ids.astype(int)] = c
    recip_res = np.concatenate([1.0 / np.maximum(cnt_res[0], 1),
                                1.0 / np.maximum(cnt_res[1], 1)]).reshape(1, 2 * N_RES).astype(f32)

    # static pair->atom one-hots (identity layout, 10 chunks of 128 pairs)
    Sseg = np.zeros((128, 10, 128), f32)
    for c in range(10):
        for j in range(128):
            Sseg[j, c, (c * 128 + j) // K] = 1.0

    Wf1 = np.asarray(inputs['Wf1'], f32)
    Wf2 = np.asarray(inputs['Wf2'], f32)
    shared = {
        'Wv': _bf(inputs['Wv']),
        'Wr': _bf(np.asarray(inputs['Wr'], f32).reshape(KB, 128, 128)
                  .transpose(1, 0, 2).reshape(128, KB * 128)),
        'Wsr1': _bf(inputs['Wsr1']), 'Wdr1': _bf(inputs['Wdr1']),
        'Wsv': _bf(inputs['Wsv']), 'Wsr2': _bf(inputs['Wsr2']),
        'Wdr2': _bf(inputs['Wdr2']),
        'Wf1t': Wf1[:128, :], 'Wf1b': Wf1[128:, :],
        'Wf2': _bf(np.concatenate([Wf2[:128], Wf2[128:]], axis=1)),
        'Wf3': _bf(np.asarray(inputs['Wf3'], f32).reshape(DF2, 1)),
        'bf1': np.asarray(inputs['bf1'], f32).reshape(2, 128).T.copy(),
        'bf2': np.asarray(inputs['bf2'], f32).reshape(DF2, 1),
        'bf3r': np.full((128, 1), float(np.asarray(inputs['bf3']).reshape(-1)[0]), f32),
        'recip_res': recip_res,
        'Sseg': _bf(Sseg.reshape(128, 10 * 128)),
        'eyeR': np.eye(MYR, dtype=f32).reshape(1, MYR * MYR),
    }
    for p, a in ((1, 'atoms1'), (2, 'atoms2')):
        hb = np.zeros((ATAB, 128), f32)
        hb[:N_ATOMS, :ATOM_CAT] = np.asarray(inputs[a], f32)
        shared[f'atomsHB_{p}'] = _bf(hb)

    per_core = []
    for core in range(NC):
        m = dict(shared)
        lo = core * LOC
        for p, (a, r, s, d, rid) in enumerate((
                ('atoms1', 'residues1', 'same1', 'diff1', 'res_ids1'),
                ('atoms2', 'residues2', 'same2', 'diff2', 'res_ids2'))):
            at = np.zeros((ATOM_CAT, LOCP), f32)
            at[:, :LOC] = np.asarray(inputs[a], f32)[lo:lo + LOC].T
            m[f'atomsT_{p + 1}'] = _bf(at)
            rt = np.zeros((BERT_DIM, LOCP), f32)
            rt[:, :LOC] = np.asarray(inputs[r], f32)[lo:lo + LOC].T
            m[f'residT_{p + 1}'] = _bf(rt.reshape(KB, 128, LOCP))
            m[f'idxA_{p + 1}'] = mk_idx(inputs[s], inputs[d], core, 0)
            m[f'idxY_{p + 1}'] = mk_idx(inputs[s], inputs[d], core, 1)
            rc = mk_recips(inputs[s], inputs[d], core)
            m[f'recips_{p + 1}'] = np.concatenate([rc[:, None, :]] * 1, 1).reshape(2, LOCP)
            rr = np.full((LOCP,), -1.0, f32)
            rr[:LOC] = np.asarray(inputs[rid], f32)[lo:lo + LOC]
            m[f'rids_{p + 1}'] = rr.reshape(AC, 128).T.copy()
        sel = np.zeros((128, RC * MYR), f32)
        for j in range(MYR):
            g = core * MYR + j
            sel[g % 128, (g // 128) * MYR + j] = 1.0
        m['sel'] = _bf(sel)
        per_core.append(m)
    return per_core


def kernel(**inputs):
    from concourse.bass_utils import run_bass_kernel_spmd
    nc = build_graph()
    in_maps = prep_inputs(inputs)
    res = run_bass_kernel_spmd(nc, in_maps, list(range(NC)))
    out = np.concatenate([np.asarray(res.results[c]['out']).reshape(-1)
                          for c in range(NC)])
    return out.astype(np.float32)
